# revision 24
# baseline (speedup 1.0000x reference)
"""AttentionPairBias kernel for 8 Trainium2 NeuronCores.

Sharding: data-parallel over (batch, query-row-block). Core c handles batch
b = c // 4 and query rows i in [(c % 4) * 128, (c % 4 + 1) * 128).

Design (~2.6x over the f32r 4-pass baseline):
  - everything bf16 (validated: rel err 8.6e-3 vs the 2e-2 budget).
  - z phase streams each (i, j) pair through the PE only twice:
      pass 1: stationary [u_centered(16) | ones | 0] on bf16 z
      pass 2: stationary [0 ... ones@17]            on bf16 z^2
    Host-centering u's columns folds the LayerNorm mean subtraction into
    the weights, so bias_h = zu_c[h] * rsig + t_h. Four i-rows pack into
    the four 32-partition PE column strips via tile_position (they execute
    concurrently); one [128, 512] evacuation covers 4 rows x 18 outputs.
  - q/k/v/g projections interleave into the z loop so the PE stays
    HAM-warm and the weight DMA overlaps the z stream.
  - DMA discipline (the hard-won part): a queue dispatches ~1 packet per
    ~50ns and a packet is one per-partition contiguous run, so
      * all small constants ride in two packed tensors (ahead/acts);
      * the z stream uses 4-group tiles (16KB runs) alternating the
        sync/scalar queues;
      * the [head,i,j]->[i,head,j] flip writes scatter (1KB runs) spread
        round-robin over all three issue engines during the loop, and the
        read-back is contiguous [i, 18KB] slices at the end, partition-
        split across the two free queues (stats rows first so the
        rsig chain starts immediately);
      * a dma_start whose semaphore wait is unmet blocks every later DMA
        on that engine -- flip DMAs are placed so waits are pre-satisfied.
  - softmax skips the max pass (|logits| <= ~9 here); exp carries the
    per-head bias t_h and accum_out row sums; P, transposes, PV, and the
    output projection all run in bf16.
"""

import sys

sys.path.insert(0, "/opt/trn_rl_repo")

from contextlib import ExitStack

import numpy as np

import concourse.bacc as bacc
import concourse.bass as bass
import concourse.mybir as mybir
import concourse.tile as tile
from concourse.bass_utils import run_bass_kernel_spmd
from concourse.masks import make_identity

F32 = mybir.dt.float32
BF16 = mybir.dt.bfloat16
AF = mybir.ActivationFunctionType
ALU = mybir.AluOpType

B, N, CS, CZ, H, D = 2, 512, 1024, 128, 16, 64
ROWS = 128          # query rows per core
NCHUNK = CS // 128  # 8 contraction chunks of 128
N_CORES = 8
EPS = 1e-5
NG = 32             # z groups (4 i-rows each); i = 32*kk + t

# acts layout (f32 columns)
A_KIN = 0           # kinT bf16 [128, 8, 512] -> 2048 f32
A_ST = 2048         # sT bf16 [128, 8, 128] -> 512 f32
A_U32 = 2560        # u32 bf16 [CZ, 32] -> 16
A_SQ32 = 2576       # sq32 bf16 [CZ, 32] -> 16
A_BQ = 2592         # bq/8 qT-layout f32 [128, 8]
A_T16 = 2600        # t bias f32 [128, 16] replicated
A_END = 2616
A_MNEG = 2616       # mask path only: f32 [128, 512]

_CACHE = {}


def _build_program(mask_trivial: bool):
    nc = bacc.Bacc("TRN2", target_bir_lowering=False, debug=False,
                   num_devices=N_CORES)

    def din(name, shape):
        return nc.dram_tensor(name, shape, F32, kind="ExternalInput").ap()

    acts_cols = A_END if mask_trivial else A_MNEG + N
    acts_d = din("acts", (128, acts_cols))  # body: kinT/sT [+mneg]
    ahead_d = din("ahead", (128, A_END - A_U32))  # u32|sq32|bq|t16
    zt_d = din("zt", (CZ, NG, 4, N // 2))        # [c, t, kk, j]  (i = 32*kk+t)
    wq_d = din("wq", (128, NCHUNK, CS // 2))     # pre-scaled by 1/8
    wk_d = din("wk", (128, NCHUNK, CS // 2))
    wv_d = din("wv", (128, NCHUNK, CS // 2))
    wg_d = din("wg", (128, NCHUNK, CS // 2))
    wo_d = din("wo", (128, NCHUNK, CS // 2))
    out_d = nc.dram_tensor("out", (ROWS, CS), F32, kind="ExternalOutput").ap()

    with tile.TileContext(nc) as tc, ExitStack() as ctx:
        dram = ctx.enter_context(tc.tile_pool(name="dram", bufs=1, space="DRAM"))
        zuF = dram.tile([128, 18, N], BF16)      # [i, h, j]

        proj = ctx.enter_context(tc.tile_pool(name="proj", bufs=1))
        ahead = proj.tile([128, A_END - A_U32], F32)
        nc.gpsimd.dma_start(ahead[:], ahead_d[:])
        acts = proj.tile([128, acts_cols], F32)
        nc.gpsimd.dma_start(acts[:], acts_d[:])

        kin_bf = (acts[:, A_KIN:A_ST].bitcast(BF16)
                  .rearrange("p (c j) -> p c j", c=NCHUNK))      # [128, 8, 512]
        sT_bf = (acts[:, A_ST:A_U32].bitcast(BF16)
                 .rearrange("p (c i) -> p c i", c=NCHUNK))       # [128, 8, 128]
        u_bf = ahead[:, 0:16].bitcast(BF16)                      # [CZ, 32]
        sq_bf = ahead[:, 16:32].bitcast(BF16)                    # [CZ, 32]
        bq8 = ahead[:, 32:40]                                    # [128, 8]
        t_b = ahead[:, 40:56]                                    # [128, 16]
        if not mask_trivial:
            mfull = acts[:, A_MNEG:A_MNEG + N]                   # [128, 512]

        # ---- weights (gpsimd SWDGE ring; wo emitted mid-loop) ----
        wpool = ctx.enter_context(tc.tile_pool(name="wpool", bufs=3))
        w_sbs = {}
        for wname, wd in [("wk", wk_d), ("wv", wv_d), ("wq", wq_d)]:
            t = wpool.tile([128, NCHUNK, CS // 2], F32, tag="w",
                           name=f"w_{wname}")
            nc.gpsimd.dma_start(t[:], wd[:])
            w_sbs[wname] = t.bitcast(BF16)       # [128, 8, 1024]

        small = ctx.enter_context(tc.tile_pool(name="small", bufs=1))
        ident = small.tile([128, 128], BF16)
        make_identity(nc, ident[:])
        ones1 = small.tile([128, 1], BF16)
        nc.vector.memset(ones1[:], 1.0)
        eps_b = small.tile([128, 1], F32)
        nc.vector.memset(eps_b[:], EPS)
        tdum = small.tile([128, 1], F32)

        # persistent projection outputs
        qT_sb = proj.tile([128, NCHUNK, ROWS], BF16)   # [d, dc, i] (q+bq)/8
        kT_sb = proj.tile([128, NCHUNK, N], BF16)      # [d, dc, j]
        v_sb = proj.tile([128, 4, CS], BF16)           # [j, jc, h*64+d]
        g_sb = proj.tile([128, CS], BF16)              # sigmoid(s @ wg)
        zu2 = proj.tile([128, 18, N], BF16)            # [i, h|musum|ss, j]

        # ------------- z loop with interleaved projections -------------
        items = []
        for dc in range(NCHUNK):
            items.append(("k", dc))
        for nh in range(2):
            for jc in range(4):
                items.append(("v", nh * 4 + jc))
        for dc in range(NCHUNK):
            items.append(("q", dc))
        for nh in range(2):
            items.append(("g", nh))
        item_at = {}
        for idx in range(len(items)):
            item_at[6 + idx] = idx

        with ExitStack() as zctx:
            # the mask-path acts tensor is 2KB/partition bigger; trade a
            # zin prefetch buffer for it there
            zinp = zctx.enter_context(tc.tile_pool(
                name="zinp", bufs=3 if mask_trivial else 2))
            zinp1 = zctx.enter_context(tc.tile_pool(name="zinp1", bufs=2))
            z2p = zctx.enter_context(tc.tile_pool(name="z2p", bufs=3))
            zstp = zctx.enter_context(tc.tile_pool(name="zstp", bufs=2))
            zstp1 = zctx.enter_context(tc.tile_pool(name="zstp1", bufs=1))
            zps = zctx.enter_context(tc.tile_pool(name="zps", bufs=2,
                                                  space="PSUM"))
            prps = zctx.enter_context(tc.tile_pool(name="prps", bufs=2,
                                                   space="PSUM"))

            ztiles = [(0, 2), (2, 2), (4, 4), (8, 4), (12, 4), (16, 4),
                      (20, 4), (24, 4), (28, 4)]
            zins = []

            def emit_ztile(zi):
                zt0, zlen = ztiles[zi]
                pool = zinp1 if zlen == 2 else zinp
                z = pool.tile([CZ, zlen, 4, N // 2], F32, tag=f"zin{zlen}",
                              name=f"zin_{zt0}")
                eng = nc.sync if zi % 2 == 0 else nc.scalar
                eng.dma_start(z[:], zt_d[:, zt0:zt0 + zlen, :, :])
                zins.append((zt0, zlen, z))

            # first five tiles issued up-front (fills every pool buffer with
            # no waits, so both queues stream from t=0); the rest are issued
            # mid-loop once their ring buffer is surely free
            for zi in range(5):
                emit_ztile(zi)
            emit_more = {12: 5, 16: 6, 20: 7, 24: 8}

            batches = [(0, 6), (6, 6), (12, 6), (18, 6), (24, 4), (28, 2),
                      (30, 1), (31, 1)]
            bat_of = {}
            for bi, (b0, blen) in enumerate(batches):
                for tt in range(b0, b0 + blen):
                    bat_of[tt] = (bi, b0, blen)
            wengs = [nc.sync, nc.scalar, nc.gpsimd]

            zu_st = None
            for t in range(NG):
                if t in emit_more:
                    emit_ztile(emit_more[t])
                for zt0, zlen, z in zins:
                    if zt0 <= t < zt0 + zlen:
                        zin, zoff = z, zt0
                        break
                bi, b0, blen = bat_of[t]
                if t == b0:
                    pool = zstp if blen == 6 else zstp1
                    zu_st = pool.tile([128, blen, N], BF16, tag=f"zst{blen}")
                zb = zin[:, t - zoff, :, :].bitcast(BF16)   # [CZ, 4(kk), 512]
                z2 = z2p.tile([CZ, 4, N], BF16, tag="z2")
                nc.scalar.activation(z2[:, 0:2, :], zb[:, 0:2, :], AF.Square)
                nc.vector.tensor_tensor(z2[:, 2:4, :], zb[:, 2:4, :],
                                        zb[:, 2:4, :], ALU.mult)
                ps = zps.tile([128, N], F32, tag="z")
                for kk in range(4):
                    tp = (0, 32 * kk)
                    dst = ps[32 * kk:32 * kk + 32, :]
                    nc.tensor.matmul(dst, u_bf[:], zb[:, kk, :],
                                     start=True, stop=False, tile_position=tp)
                    nc.tensor.matmul(dst, sq_bf[:], z2[:, kk, :],
                                     start=False, stop=True, tile_position=tp)
                nc.vector.tensor_copy(zu_st[:, t - b0, :], ps[:])
                if t == b0 + blen - 1:
                    for kk in range(4):
                        weng = wengs[(4 * bi + kk) % 3]
                        weng.dma_start(
                            zuF[32 * kk + b0:32 * kk + b0 + blen, :, :]
                            .rearrange("t h j -> h t j"),
                            zu_st[32 * kk:32 * kk + 18, :, :])

                if t == 16:
                    wg_t = wpool.tile([128, NCHUNK, CS // 2], F32, tag="w",
                                      name="w_wg")
                    nc.gpsimd.dma_start(wg_t[:], wg_d[:])
                    w_sbs["wg"] = wg_t.bitcast(BF16)
                if t == 24:
                    wo_t = wpool.tile([128, NCHUNK, CS // 2], F32, tag="w",
                                      name="w_wo")
                    nc.gpsimd.dma_start(wo_t[:], wo_d[:])
                    w_sbs["wo"] = wo_t.bitcast(BF16)

                it = item_at.get(t)
                if it is None:
                    continue
                kind, a = items[it]
                if kind == "k":
                    dc = a
                    ps2 = prps.tile([128, N], F32, tag="pk")
                    for cc in range(NCHUNK):
                        nc.tensor.matmul(
                            ps2[:], w_sbs["wk"][:, cc, 128 * dc:128 * dc + 128],
                            kin_bf[:, cc, :],
                            start=(cc == 0), stop=(cc == NCHUNK - 1))
                    nc.scalar.copy(kT_sb[:, dc, :], ps2[:])
                elif kind == "v":
                    nh, jc = a // 4, a % 4
                    ps2 = prps.tile([128, N], F32, tag="pk")
                    for cc in range(NCHUNK):
                        nc.tensor.matmul(
                            ps2[:], kin_bf[:, cc, 128 * jc:128 * jc + 128],
                            w_sbs["wv"][:, cc, 512 * nh:512 * nh + 512],
                            start=(cc == 0), stop=(cc == NCHUNK - 1))
                    nc.scalar.copy(v_sb[:, jc, 512 * nh:512 * nh + 512], ps2[:])
                elif kind == "q":
                    dc = a
                    ps2 = prps.tile([128, ROWS], F32, tag="pq")
                    for cc in range(NCHUNK):
                        nc.tensor.matmul(
                            ps2[:], w_sbs["wq"][:, cc, 128 * dc:128 * dc + 128],
                            sT_bf[:, cc, :],
                            start=(cc == 0), stop=(cc == NCHUNK - 1))
                    nc.vector.tensor_scalar_add(qT_sb[:, dc, :], ps2[:],
                                                bq8[:, dc:dc + 1])
                else:  # g
                    nh = a
                    ps2 = prps.tile([128, N], F32, tag="pk")
                    for cc in range(NCHUNK):
                        nc.tensor.matmul(
                            ps2[:], sT_bf[:, cc, :],
                            w_sbs["wg"][:, cc, 512 * nh:512 * nh + 512],
                            start=(cc == 0), stop=(cc == NCHUNK - 1))
                    nc.scalar.activation(g_sb[:, 512 * nh:512 * nh + 512],
                                         ps2[:], AF.Sigmoid)

            # preload the Sqrt activation table while the loop drains
            nc.scalar.activation(tdum[:], eps_b[:], AF.Sqrt)

            # contiguous read-back, partition-split across both free
            # queues (keeps per-partition runs contiguous): stats rows
            # first for the rsig chain, then head quads
            nc.sync.dma_start(zu2[0:64, 16:18, :], zuF[0:64, 16:18, :])
            nc.scalar.dma_start(zu2[64:128, 16:18, :], zuF[64:128, 16:18, :])
            for hc in range(4):
                nc.sync.dma_start(zu2[0:64, 4 * hc:4 * hc + 4, :],
                                  zuF[0:64, 4 * hc:4 * hc + 4, :])
                nc.scalar.dma_start(zu2[64:128, 4 * hc:4 * hc + 4, :],
                                    zuF[64:128, 4 * hc:4 * hc + 4, :])

        # ------------- rsig, per-head bias -------------
        apool = ctx.enter_context(tc.tile_pool(name="apool", bufs=1))
        att = ctx.enter_context(tc.tile_pool(name="att", bufs=3))
        spsum = ctx.enter_context(tc.tile_pool(name="spsum", bufs=3, space="PSUM"))
        tpsum = ctx.enter_context(tc.tile_pool(name="tpsum", bufs=2, space="PSUM"))
        opsum = ctx.enter_context(tc.tile_pool(name="opsum", bufs=2, space="PSUM"))
        kaps = ctx.enter_context(tc.tile_pool(name="kaps", bufs=1, space="PSUM"))

        ka_ps = kaps.tile([128, 128], BF16, tag="ka")
        m2 = apool.tile([128, N], F32)
        nc.tensor.transpose(ka_ps[:], zu2[:, 16, 0:128], ident[:])
        nc.vector.tensor_tensor(m2[:], zu2[:, 16, :], zu2[:, 16, :], ALU.mult)
        wvar = apool.tile([128, N], F32)   # 128 * var
        nc.vector.scalar_tensor_tensor(wvar[:], m2[:], -1.0 / CZ,
                                       zu2[:, 17, :], op0=ALU.mult, op1=ALU.add)
        sdev = apool.tile([128, N], F32)
        nc.scalar.activation(sdev[:], wvar[:], AF.Sqrt, bias=eps_b[:, 0:1],
                             scale=1.0 / CZ)
        # preload the Exp table while the reciprocal runs
        nc.scalar.activation(tdum[:], eps_b[:], AF.Exp)
        rsig = apool.tile([128, N], F32)
        nc.tensor.transpose(ka_ps[:], zu2[:, 17, 0:128], ident[:])
        nc.vector.reciprocal(rsig[:, 0:N // 2], sdev[:, 0:N // 2])
        nc.vector.reciprocal(rsig[:, N // 2:N], sdev[:, N // 2:N])
        nc.tensor.transpose(ka_ps[:], zu2[:, 16, 128:256], ident[:])

        # per-head bias tiles on the (otherwise idle) gpsimd engine,
        # staggered on the quad-read arrivals
        biasAll = apool.tile([128, H, N], F32)
        for hc in range(8):
            h0 = 2 * hc
            nc.gpsimd.tensor_tensor(
                biasAll[:, h0:h0 + 2, :], zu2[:, h0:h0 + 2, :],
                rsig[:, None, :].to_broadcast([128, 2, N]), ALU.mult)
            if not mask_trivial:
                nc.gpsimd.tensor_tensor(
                    biasAll[:, h0:h0 + 2, :], biasAll[:, h0:h0 + 2, :],
                    mfull[:, None, :].to_broadcast([128, 2, N]), ALU.add)

        o_all = apool.tile([128, H, D], F32)
        sums = apool.tile([128, H], F32)

        # ------------- attention, one head at a time -------------
        for h in range(H):
            p0 = 64 * (h % 2)
            sc = spsum.tile([128, N], F32, tag="sc")
            nc.tensor.matmul(sc[:], qT_sb[p0:p0 + 64, h // 2, :],
                             kT_sb[p0:p0 + 64, h // 2, :],
                             start=True, stop=True)
            nc.vector.tensor_tensor(sc[:], sc[:], biasAll[:, h, :], ALU.add)
            p_sb = att.tile([128, N], BF16, tag="p")
            nc.scalar.activation(p_sb[:], sc[:], AF.Exp, bias=t_b[:, h:h + 1],
                                 accum_out=sums[:, h:h + 1])
            pt_ps = tpsum.tile([128, N], BF16, tag="pt")
            for jc in range(4):
                nc.tensor.transpose(pt_ps[:, 128 * jc:128 * jc + 128],
                                    p_sb[:, 128 * jc:128 * jc + 128], ident[:])
            pt_sb = att.tile([128, N], BF16, tag="ptsb")
            if h % 2 == 0:
                nc.scalar.copy(pt_sb[:], pt_ps[:])
            else:
                nc.vector.tensor_copy(pt_sb[:], pt_ps[:])
            o_ps = opsum.tile([128, D], F32, tag="o")
            for jc in range(4):
                nc.tensor.matmul(o_ps[:],
                                 pt_sb[:, 128 * jc:128 * jc + 128],
                                 v_sb[:, jc, D * h:D * h + D],
                                 start=(jc == 0), stop=(jc == 3))
            nc.vector.tensor_copy(o_all[:, h, :], o_ps[:])

        # ------------- gate, transpose, output projection -------------
        recip = apool.tile([128, H], F32)
        nc.vector.reciprocal(recip[:], sums[:])
        go = apool.tile([128, H, D], BF16)
        nc.vector.tensor_tensor(go[:], o_all[:],
                                recip[:, :, None].to_broadcast([128, H, D]),
                                ALU.mult)
        gof = go.rearrange("p h d -> p (h d)")
        go2 = apool.tile([128, CS], BF16)
        nc.vector.tensor_tensor(go2[:], gof[:], g_sb[:], ALU.mult)

        goT = apool.tile([128, NCHUNK, ROWS], BF16)
        for half in range(2):
            gt_ps = tpsum.tile([128, N], BF16, tag="pt")
            for j4 in range(4):
                cc = 4 * half + j4
                nc.tensor.transpose(gt_ps[:, 128 * j4:128 * j4 + 128],
                                    go2[:, 128 * cc:128 * cc + 128], ident[:])
            if half == 0:
                nc.scalar.copy(goT[:, 0:4, :], gt_ps[:])
            else:
                nc.vector.tensor_copy(goT[:, 4:8, :], gt_ps[:])

        out_sb = apool.tile([128, CS], F32)
        for nh in range(2):
            ps3 = spsum.tile([128, N], F32, tag="sc")
            for cc in range(NCHUNK):
                nc.tensor.matmul(ps3[:], goT[:, cc, :],
                                 w_sbs["wo"][:, cc, 512 * nh:512 * nh + 512],
                                 start=(cc == 0), stop=(cc == NCHUNK - 1))
            nc.vector.tensor_copy(out_sb[:, 512 * nh:512 * nh + 512], ps3[:])
        nc.sync.dma_start(out_d[:], out_sb[:])

    nc.compile()
    return nc


def _prepare(s, z, mask, k_in, wq, bq, wk, wv, wg, ln_g, ln_b, wz, wo,
             multiplicity=1, **_ignored):
    import ml_dtypes
    bf = ml_dtypes.bfloat16
    s = np.asarray(s, dtype=np.float32)
    z = np.asarray(z, dtype=np.float32)
    mask = np.asarray(mask, dtype=np.float32)
    k_in = np.asarray(k_in, dtype=np.float32)
    assert int(multiplicity) == 1, "only multiplicity == 1 is supported"
    mask_trivial = bool(np.all(mask == 1.0))
    acts_cols = A_END if mask_trivial else A_MNEG + N

    def wchunk(w):
        # [1024, 1024] f32 -> [128, 8, 1024] bf16 -> f32-packed [128, 8, 512]
        return np.ascontiguousarray(
            np.asarray(w, dtype=np.float32).reshape(NCHUNK, 128, CS)
            .transpose(1, 0, 2).astype(bf)).view(np.float32)

    u = np.asarray(ln_g, np.float32)[:, None] * np.asarray(wz, np.float32)
    uc = u - u.mean(axis=0, keepdims=True)
    u32 = np.zeros((CZ, 32), dtype=bf)
    u32[:, 0:H] = uc.astype(bf)
    u32[:, H] = 1.0
    sq32 = np.zeros((CZ, 32), dtype=bf)
    sq32[:, 17] = 1.0
    t16 = (np.asarray(ln_b, np.float32) @ np.asarray(wz, np.float32))

    shared = {
        "wq": wchunk(np.asarray(wq, np.float32) / 8.0),
        "wk": wchunk(wk), "wv": wchunk(wv), "wg": wchunk(wg),
        "wo": wchunk(wo),
    }
    ahead = np.zeros((128, A_END - A_U32), dtype=np.float32)
    ahead[:, 0:16] = u32.view(np.float32)
    ahead[:, 16:32] = sq32.view(np.float32)
    ahead[:, 32:40] = (
        (np.asarray(bq, dtype=np.float32) / 8.0).reshape(NCHUNK, 128).T)
    ahead[:, 40:56] = t16.reshape(1, H)
    shared["ahead"] = ahead
    acts_common = np.zeros((128, acts_cols), dtype=np.float32)

    in_maps = []
    for core in range(N_CORES):
        b, ib = core // 4, core % 4
        i0 = ib * ROWS
        m = dict(shared)
        a = acts_common.copy()
        a[:, A_KIN:A_ST] = (
            k_in[b].T.reshape(NCHUNK, 128, N).transpose(1, 0, 2)
            .astype(bf).reshape(128, -1).view(np.float32))
        a[:, A_ST:A_U32] = (
            s[b, i0:i0 + ROWS, :].T.reshape(NCHUNK, 128, ROWS)
            .transpose(1, 0, 2).astype(bf).reshape(128, -1).view(np.float32))
        if not mask_trivial:
            a[:, A_MNEG:A_MNEG + N] = ((1.0 - mask[b]) * -1e6).reshape(1, N)
        m["acts"] = a
        # z -> [c, t, kk, j] with i = 32*kk + t, bf16
        zt = (z[b, i0:i0 + ROWS].transpose(2, 0, 1)          # [c, i, j]
              .reshape(CZ, 4, NG, N).transpose(0, 2, 1, 3))  # [c, t, kk, j]
        m["zt"] = np.ascontiguousarray(zt.astype(bf)).view(np.float32)
        in_maps.append(m)
    return mask_trivial, in_maps


def _run(in_maps, mask_trivial, **kwargs):
    if mask_trivial not in _CACHE:
        _CACHE[mask_trivial] = _build_program(mask_trivial)
    nc = _CACHE[mask_trivial]
    res = run_bass_kernel_spmd(nc, in_maps, core_ids=list(range(N_CORES)),
                               **kwargs)
    out = np.empty((B, N, CS), dtype=np.float32)
    for core in range(N_CORES):
        b, ib = core // 4, core % 4
        out[b, ib * ROWS:(ib + 1) * ROWS, :] = res.results[core]["out"]
    return out, res


def kernel(**inputs):
    mask_trivial, in_maps = _prepare(**inputs)
    out, _ = _run(in_maps, mask_trivial)
    return out


def run_profiled(inputs, tmpdir=None):
    mask_trivial, in_maps = _prepare(**inputs)
    out, res = _run(in_maps, mask_trivial, trace=True, tmpdir=tmpdir)
    return out, res
